# revision 11
# baseline (speedup 1.0000x reference)
"""ConvNearestNeightbor Trainium2 kernel.

out[b, n*C+c, i, j] = max_k |x[b,c,i-r_k,j-c_k] - neighbors[n,c,k]|
over the 9 zero-padded 3x3 shifts (r_k, c_k).

Sharding: 8 cores = 4 batch-groups x 2 num-groups.
Per core: B_loc=4 batches, N_loc=16 codebook entries.
Partition layout: (nn in 0..3, c in 0..31) -> 128 partitions, with the
codebook tile index nt in 0..3 selecting n = nt*4+nn.
Free dim per op: (b in 0..3, 32x32 pixels) = 4096.

Per (nt): 9 abs-diff planes d_k = |x_win_k - nb_k| are produced
(mostly on ScalarE via Abs activation with per-partition bias -nb;
optionally a few on VectorE via tensor_scalar subtract + bitwise-and
sign clear), then folded with tensor_tensor max on VectorE.
PREC="fp16" keeps d/acc in fp16 (2x DVE fold rate, one extra rounding
of ~2^-11 relative); PREC="fp32" is bit-exact vs the fp32 reference.
"""

import numpy as np

B, C, H, W = 16, 32, 32, 32
NUM = 32
NCORES = 8
BG, NG = 4, 2          # batch groups x num groups
B_LOC = B // BG        # 4
N_LOC = NUM // NG      # 16
NT = N_LOC // 4        # 4 codebook tiles of 4 n each
PH, PW = H + 2, W + 2  # 34 x 34 padded image
FREE = B_LOC * H * W   # 4096

PREC = "fp16"          # "fp16" or "fp32"
# per-nt production placement: VectorE and Pool take these shifts,
# ScalarE (Abs+bias) takes the rest. Alternating DVE share balances
# ACT vs DVE totals.
DVE_PROD_KS = {0: (0, 2), 1: (0,), 2: (0, 2), 3: (0,)}
POOL_PROD_KS = {0: (4,), 1: (4,), 2: (4,), 3: (4,)}

_module_cache = {}


def _build_module():
    import concourse.bacc as bacc
    import concourse.mybir as mybir
    import concourse.tile as tile

    dt = mybir.dt
    Alu = mybir.AluOpType
    AF = mybir.ActivationFunctionType

    cdt = dt.float16 if PREC == "fp16" else dt.float32
    idt = dt.uint16 if PREC == "fp16" else dt.uint32
    mask = 0x7FFF if PREC == "fp16" else 0x7FFFFFFF

    nc = bacc.Bacc("TRN2", debug=False)
    x = nc.dram_tensor("x", [B_LOC, C, H, W], dt.float32, kind="ExternalInput")
    nb = nc.dram_tensor("neighbors", [N_LOC, C, 9], dt.float32, kind="ExternalInput")
    out = nc.dram_tensor(
        "out", [B_LOC, N_LOC * C, H, W], dt.float32, kind="ExternalOutput"
    )

    # window start offsets within the padded 34x34 image for the 9 shifts
    # k = (row+1)*3 + (col+1), window starts at (1-row, 1-col)
    offs = []
    for row in (-1, 0, 1):
        for col in (-1, 0, 1):
            offs.append((1 - row, 1 - col))

    with tile.TileContext(nc) as tc:
        with (
            tc.tile_pool(name="const", bufs=1) as cpool,
            tc.tile_pool(name="accp", bufs=3) as apool,
            tc.tile_pool(name="dp", bufs=6) as dpool,
        ):
            nbt = cpool.tile([128, NT * 9], dt.float32, tag="nbt")
            # nbt[(nn,c), (t,k)] = neighbors[t*4+nn, c, k]
            nb_src = nb.ap().rearrange("(t nn) c k -> (nn c) t k", nn=4)
            nbt_v = nbt[:].rearrange("p (t k) -> p t k", t=NT)
            nc.sync.dma_start(nbt_v, nb_src)
            # negated neighbors: ACT bias computes Abs(x + (-nb))
            nbneg = cpool.tile([128, NT * 9], dt.float32, tag="nbneg")
            nc.scalar.mul(nbneg[:], nbt[:], -1.0)

            # raw x load (contiguous, fast descriptors), kept f32
            xraw = cpool.tile([128, B_LOC * H * W], dt.float32, tag="xraw")
            xraw_v = xraw[:].rearrange("p (b h w) -> p b h w", b=B_LOC, h=H, w=W)
            x_src = x.ap().rearrange("b c h w -> c b h w")
            for nn in range(4):
                dst = xraw_v[nn * 32 : (nn + 1) * 32]
                nc.sync.dma_start(
                    dst.rearrange("c b h w -> c b (h w)"),
                    x_src.rearrange("c b h w -> c b (h w)"),
                )

            # padded f32 image, borders zero; interior copied on GpSimd
            xpad = cpool.tile([128, B_LOC * PH * PW], dt.float32, tag="xpad")
            nc.vector.memset(xpad[:], 0.0)
            xpad_v = xpad[:].rearrange("p (b h w) -> p b h w", b=B_LOC, h=PH, w=PW)
            nc.gpsimd.tensor_copy(xpad_v[:, :, 1 : 1 + H, 1 : 1 + W], xraw_v)

            out_v = out.ap().rearrange(
                "b (t p) h w -> t p b (h w)", t=NT
            )  # p = 128 partition-channels per tile

            for nt in range(NT):
                acc = apool.tile([128, FREE], cdt, tag="acc")
                nfold = 0
                first = None  # first produced d tile, folded on second
                for k in range(9):
                    a, bcol = offs[k]
                    xwin = xpad_v[:, :, a : a + H, bcol : bcol + W]
                    d = dpool.tile([128, FREE], cdt, tag="d")
                    d_v = d[:].rearrange("p (b h w) -> p b h w", b=B_LOC, h=H, w=W)
                    if k in DVE_PROD_KS[nt]:
                        # d = x_win - nb ; then clear sign bit -> |d|
                        nc.vector.tensor_scalar(
                            d_v, xwin, nbt[:, nt * 9 + k : nt * 9 + k + 1], None,
                            Alu.subtract,
                        )
                        nc.vector.tensor_scalar(
                            d[:].bitcast(idt), d[:].bitcast(idt), mask, None,
                            Alu.bitwise_and,
                        )
                    elif k in POOL_PROD_KS[nt]:
                        # Pool computes the signed diff; DVE clears the sign
                        nc.gpsimd.tensor_scalar(
                            d_v, xwin, nbt[:, nt * 9 + k : nt * 9 + k + 1], None,
                            Alu.subtract,
                        )
                        nc.vector.tensor_scalar(
                            d[:].bitcast(idt), d[:].bitcast(idt), mask, None,
                            Alu.bitwise_and,
                        )
                    else:
                        # d = |x_win + (-nb)| on ScalarE
                        nc.scalar.activation(
                            d_v, xwin, AF.Abs,
                            bias=nbneg[:, nt * 9 + k : nt * 9 + k + 1], scale=1.0,
                        )
                    if first is None:
                        first = d
                    elif nfold == 0:
                        nc.vector.tensor_tensor(acc[:], first[:], d[:], Alu.max)
                        nfold = 1
                    else:
                        nc.vector.tensor_tensor(acc[:], acc[:], d[:], Alu.max)
                acc_s = acc[:].rearrange("p (b s) -> p b s", b=B_LOC)
                if PREC == "fp16":
                    nc.gpsimd.dma_start(out_v[nt], acc_s)  # SWDGE cast fp16->f32
                else:
                    nc.sync.dma_start(out_v[nt], acc_s)

    nc.compile()
    return nc


def _get_module():
    if "nc" not in _module_cache:
        _module_cache["nc"] = _build_module()
    return _module_cache["nc"]


def _run(x, neighbors, trace=False):
    from concourse import bass_utils

    x = np.ascontiguousarray(x, dtype=np.float32)
    neighbors = np.ascontiguousarray(neighbors, dtype=np.float32)
    in_maps = []
    for core in range(NCORES):
        bg, ng = divmod(core, NG)
        in_maps.append(
            {
                "x": x[bg * B_LOC : (bg + 1) * B_LOC],
                "neighbors": neighbors[ng * N_LOC : (ng + 1) * N_LOC],
            }
        )
    res = bass_utils.run_bass_kernel_spmd(
        _get_module(), in_maps, core_ids=list(range(NCORES)), trace=trace
    )
    out = np.empty((B, NUM * C, H, W), dtype=np.float32)
    for core in range(NCORES):
        bg, ng = divmod(core, NG)
        out[bg * B_LOC : (bg + 1) * B_LOC, ng * N_LOC * C : (ng + 1) * N_LOC * C] = (
            res.results[core]["out"]
        )
    return out, res


def kernel(x, neighbors):
    out, _ = _run(x, neighbors, trace=False)
    return out


# revision 13
# speedup vs baseline: 2.3327x; 2.3327x over previous
"""ConvNearestNeightbor Trainium2 kernel.

out[b, n*C+c, i, j] = max_k |x[b,c,i-r_k,j-c_k] - neighbors[n,c,k]|
over the 9 zero-padded 3x3 shifts (r_k, c_k).

Sharding: 8 cores = 4 batch-groups x 2 num-groups.
Per core: B_loc=4 batches, N_loc=16 codebook entries.
Partition layout: (nn in 0..3, c in 0..31) -> 128 partitions, with the
codebook tile index nt in 0..3 selecting n = nt*4+nn.
Free dim per op: (b in 0..3, 32x32 pixels) = 4096.

Per (nt): 9 abs-diff planes d_k = |x_win_k - nb_k| are produced
(mostly on ScalarE via Abs activation with per-partition bias -nb;
optionally a few on VectorE via tensor_scalar subtract + bitwise-and
sign clear), then folded with tensor_tensor max on VectorE.
PREC="fp16" keeps d/acc in fp16 (2x DVE fold rate, one extra rounding
of ~2^-11 relative); PREC="fp32" is bit-exact vs the fp32 reference.
"""

import numpy as np

B, C, H, W = 16, 32, 32, 32
NUM = 32
NCORES = 8
BG, NG = 4, 2          # batch groups x num groups
B_LOC = B // BG        # 4
N_LOC = NUM // NG      # 16
NT = N_LOC // 4        # 4 codebook tiles of 4 n each
PH, PW = H + 2, W + 2  # 34 x 34 padded image
FREE = B_LOC * H * W   # 4096

PREC = "fp16"          # "fp16" or "fp32"
# per-nt production placement: VectorE takes these shifts (tensor_scalar
# subtract + bitwise-and sign clear), ScalarE (Abs+bias) takes the rest.
# Fractional balance via per-nt variation. Window offsets for these k must
# be 4B-aligned for fp16 4x mode: k in {0,2,3,5,6,8}.
DVE_PROD_KS = {0: (0, 2, 6), 1: (0, 2), 2: (0, 2, 6), 3: (0, 2, 6)}
POOL_PROD_KS = {0: (), 1: (), 2: (), 3: ()}

_module_cache = {}


def _build_module():
    import concourse.bacc as bacc
    import concourse.mybir as mybir
    import concourse.tile as tile

    dt = mybir.dt
    Alu = mybir.AluOpType
    AF = mybir.ActivationFunctionType

    cdt = dt.float16 if PREC == "fp16" else dt.float32
    idt = dt.uint16 if PREC == "fp16" else dt.uint32
    mask = 0x7FFF if PREC == "fp16" else 0x7FFFFFFF

    nc = bacc.Bacc("TRN2", debug=False)
    x = nc.dram_tensor("x", [B_LOC, C, H, W], dt.float32, kind="ExternalInput")
    nb = nc.dram_tensor("neighbors", [N_LOC, C, 9], dt.float32, kind="ExternalInput")
    out = nc.dram_tensor(
        "out", [B_LOC, N_LOC * C, H, W], dt.float32, kind="ExternalOutput"
    )

    # window start offsets within the padded 34x34 image for the 9 shifts
    # k = (row+1)*3 + (col+1), window starts at (1-row, 1-col)
    offs = []
    for row in (-1, 0, 1):
        for col in (-1, 0, 1):
            offs.append((1 - row, 1 - col))

    with tile.TileContext(nc) as tc:
        with (
            tc.tile_pool(name="const", bufs=1) as cpool,
            tc.tile_pool(name="accp", bufs=3) as apool,
            tc.tile_pool(name="dp", bufs=6) as dpool,
        ):
            nbt = cpool.tile([128, NT * 9], dt.float32, tag="nbt")
            # nbt[(nn,c), (t,k)] = neighbors[t*4+nn, c, k]
            nb_src = nb.ap().rearrange("(t nn) c k -> (nn c) t k", nn=4)
            nbt_v = nbt[:].rearrange("p (t k) -> p t k", t=NT)
            nc.sync.dma_start(nbt_v, nb_src)
            # negated neighbors: ACT bias computes Abs(x + (-nb))
            nbneg = cpool.tile([128, NT * 9], dt.float32, tag="nbneg")
            nc.scalar.mul(nbneg[:], nbt[:], -1.0)

            # raw x load (contiguous, fast descriptors), kept f32
            xraw = cpool.tile([128, B_LOC * H * W], dt.float32, tag="xraw")
            xraw_v = xraw[:].rearrange("p (b h w) -> p b h w", b=B_LOC, h=H, w=W)
            x_src = x.ap().rearrange("b c h w -> c b h w")
            for nn in range(4):
                dst = xraw_v[nn * 32 : (nn + 1) * 32]
                nc.sync.dma_start(
                    dst.rearrange("c b h w -> c b (h w)"),
                    x_src.rearrange("c b h w -> c b (h w)"),
                )

            # padded image in compute dtype, borders zero; interior
            # cast-copied on ScalarE (GpSimd strided ops are very slow)
            xpad = cpool.tile([128, B_LOC * PH * PW], cdt, tag="xpad")
            nc.vector.memset(xpad[:], 0.0)
            xpad_v = xpad[:].rearrange("p (b h w) -> p b h w", b=B_LOC, h=PH, w=PW)
            nc.scalar.copy(xpad_v[:, :, 1 : 1 + H, 1 : 1 + W], xraw_v)

            out_v = out.ap().rearrange(
                "b (t p) h w -> t p b (h w)", t=NT
            )  # p = 128 partition-channels per tile

            for nt in range(NT):
                acc = apool.tile([128, FREE], cdt, tag="acc")
                nfold = 0
                first = None  # first produced d tile, folded on second
                for k in range(9):
                    a, bcol = offs[k]
                    xwin = xpad_v[:, :, a : a + H, bcol : bcol + W]
                    d = dpool.tile([128, FREE], cdt, tag="d")
                    d_v = d[:].rearrange("p (b h w) -> p b h w", b=B_LOC, h=H, w=W)
                    if k in DVE_PROD_KS[nt]:
                        # d = x_win - nb ; then clear sign bit -> |d|
                        nc.vector.tensor_scalar(
                            d_v, xwin, nbt[:, nt * 9 + k : nt * 9 + k + 1], None,
                            Alu.subtract,
                        )
                        nc.vector.tensor_scalar(
                            d[:].bitcast(idt), d[:].bitcast(idt), mask, None,
                            Alu.bitwise_and,
                        )
                    elif k in POOL_PROD_KS[nt]:
                        # Pool computes the signed diff; DVE clears the sign
                        nc.gpsimd.tensor_scalar(
                            d_v, xwin, nbt[:, nt * 9 + k : nt * 9 + k + 1], None,
                            Alu.subtract,
                        )
                        nc.vector.tensor_scalar(
                            d[:].bitcast(idt), d[:].bitcast(idt), mask, None,
                            Alu.bitwise_and,
                        )
                    else:
                        # d = |x_win + (-nb)| on ScalarE
                        nc.scalar.activation(
                            d_v, xwin, AF.Abs,
                            bias=nbneg[:, nt * 9 + k : nt * 9 + k + 1], scale=1.0,
                        )
                    if first is None:
                        first = d
                    elif nfold == 0:
                        nc.vector.tensor_tensor(acc[:], first[:], d[:], Alu.max)
                        nfold = 1
                    else:
                        nc.vector.tensor_tensor(acc[:], acc[:], d[:], Alu.max)
                acc_s = acc[:].rearrange("p (b s) -> p b s", b=B_LOC)
                if PREC == "fp16":
                    nc.gpsimd.dma_start(out_v[nt], acc_s)  # SWDGE cast fp16->f32
                else:
                    nc.sync.dma_start(out_v[nt], acc_s)

    nc.compile()
    return nc


def _get_module():
    if "nc" not in _module_cache:
        _module_cache["nc"] = _build_module()
    return _module_cache["nc"]


def _run(x, neighbors, trace=False):
    from concourse import bass_utils

    x = np.ascontiguousarray(x, dtype=np.float32)
    neighbors = np.ascontiguousarray(neighbors, dtype=np.float32)
    in_maps = []
    for core in range(NCORES):
        bg, ng = divmod(core, NG)
        in_maps.append(
            {
                "x": x[bg * B_LOC : (bg + 1) * B_LOC],
                "neighbors": neighbors[ng * N_LOC : (ng + 1) * N_LOC],
            }
        )
    res = bass_utils.run_bass_kernel_spmd(
        _get_module(), in_maps, core_ids=list(range(NCORES)), trace=trace
    )
    out = np.empty((B, NUM * C, H, W), dtype=np.float32)
    for core in range(NCORES):
        bg, ng = divmod(core, NG)
        out[bg * B_LOC : (bg + 1) * B_LOC, ng * N_LOC * C : (ng + 1) * N_LOC * C] = (
            res.results[core]["out"]
        )
    return out, res


def kernel(x, neighbors):
    out, _ = _run(x, neighbors, trace=False)
    return out


# revision 14
# speedup vs baseline: 2.7312x; 1.1708x over previous
"""ConvNearestNeightbor Trainium2 kernel.

out[b, n*C+c, i, j] = max_k |x[b,c,i-r_k,j-c_k] - neighbors[n,c,k]|
over the 9 zero-padded 3x3 shifts (r_k, c_k).

Sharding: 8 cores = 4 batch-groups x 2 num-groups.
Per core: B_loc=4 batches, N_loc=16 codebook entries.
Partition layout: (nn in 0..3, c in 0..31) -> 128 partitions, with the
codebook tile index nt in 0..3 selecting n = nt*4+nn.
Work is split into batch-halves: each (nt, half) chain runs ops of
free size 2*32*32 = 2048 (b-pair, pixels).

Per chain: 9 abs-diff planes d_k = |x_win_k - nb_k| are produced
(ScalarE Abs activation with per-partition bias -nb for most k;
VectorE tensor_scalar subtract + bitwise-and sign clear for a few),
then folded with tensor_tensor max on VectorE.
PREC="fp16" keeps d/acc in fp16 (2x DVE fold rate, one fp16 rounding
~2^-11 relative); PREC="fp32" is bit-exact vs the fp32 reference.
"""

import numpy as np

B, C, H, W = 16, 32, 32, 32
NUM = 32
NCORES = 8
BG, NG = 4, 2          # batch groups x num groups
B_LOC = B // BG        # 4
N_LOC = NUM // NG      # 16
NT = N_LOC // 4        # 4 codebook tiles of 4 n each
PH, PW = H + 2, W + 2  # 34 x 34 padded image
HB = B_LOC // 2        # batches per half-chain
FREE = HB * H * W      # 2048 per half

PREC = "fp16"          # "fp16" or "fp32"
# shifts produced on VectorE (tensor_scalar subtract + bitwise-and sign
# clear); ScalarE (Abs+bias) takes the rest. Window offsets for DVE k's
# should be 4B-aligned for fp16 4x mode: k in {0,2,3,5,6,8}.
DVE_PROD_KS = (0, 2, 6)

_module_cache = {}


def _build_module():
    import concourse.bacc as bacc
    import concourse.mybir as mybir
    import concourse.tile as tile

    dt = mybir.dt
    Alu = mybir.AluOpType
    AF = mybir.ActivationFunctionType

    cdt = dt.float16 if PREC == "fp16" else dt.float32
    idt = dt.uint16 if PREC == "fp16" else dt.uint32
    mask = 0x7FFF if PREC == "fp16" else 0x7FFFFFFF

    nc = bacc.Bacc("TRN2", debug=False)
    x = nc.dram_tensor("x", [B_LOC, C, H, W], dt.float32, kind="ExternalInput")
    nb = nc.dram_tensor("neighbors", [N_LOC, C, 9], dt.float32, kind="ExternalInput")
    out = nc.dram_tensor(
        "out", [B_LOC, N_LOC * C, H, W], dt.float32, kind="ExternalOutput"
    )

    # window start offsets within the padded 34x34 image for the 9 shifts
    # k = (row+1)*3 + (col+1), window starts at (1-row, 1-col)
    offs = []
    for row in (-1, 0, 1):
        for col in (-1, 0, 1):
            offs.append((1 - row, 1 - col))

    with tile.TileContext(nc) as tc:
        with (
            tc.tile_pool(name="const", bufs=1) as cpool,
            tc.tile_pool(name="accp", bufs=4) as apool,
            tc.tile_pool(name="dp", bufs=8) as dpool,
        ):
            # raw x halves (contiguous loads on both HWDGE queues)
            x_src = x.ap().rearrange("b c h w -> c b h w")
            xraw = []
            for h in range(2):
                t = cpool.tile([128, FREE], dt.float32, tag=f"xraw{h}")
                tv = t[:].rearrange("p (b h w) -> p b h w", b=HB, h=H, w=W)
                for nn in range(4):
                    eng = nc.sync if nn % 2 == 0 else nc.scalar
                    eng.dma_start(
                        tv[nn * 32 : (nn + 1) * 32].rearrange("c b h w -> c b (h w)"),
                        x_src[:, h * HB : (h + 1) * HB].rearrange(
                            "c b h w -> c b (h w)"
                        ),
                    )
                xraw.append(tv)

            # padded halves, borders zero; interior cast-copied on ScalarE
            xpad = []
            for h in range(2):
                t = cpool.tile([128, HB * PH * PW], cdt, tag=f"xpad{h}")
                nc.vector.memset(t[:], 0.0)
                tv = t[:].rearrange("p (b h w) -> p b h w", b=HB, h=PH, w=PW)
                nc.scalar.copy(tv[:, :, 1 : 1 + H, 1 : 1 + W], xraw[h])
                xpad.append(tv)

            nbt = cpool.tile([128, NT * 9], dt.float32, tag="nbt")
            # nbt[(nn,c), (t,k)] = neighbors[t*4+nn, c, k]
            nb_src = nb.ap().rearrange("(t nn) c k -> (nn c) t k", nn=4)
            nbt_v = nbt[:].rearrange("p (t k) -> p t k", t=NT)
            nc.gpsimd.dma_start(nbt_v, nb_src)
            # negated neighbors: ACT bias computes Abs(x + (-nb))
            nbneg = cpool.tile([128, NT * 9], dt.float32, tag="nbneg")
            nc.scalar.mul(nbneg[:], nbt[:], -1.0)

            # out[b, nt*128 + p, h, w] viewed per (nt, half)
            out_v = out.ap().rearrange("b (t p) h w -> t p b (h w)", t=NT)

            for nt in range(NT):
                for h in range(2):
                    acc = apool.tile([128, FREE], cdt, tag="acc")
                    nfold = 0
                    first = None
                    for k in range(9):
                        a, bcol = offs[k]
                        xwin = xpad[h][:, :, a : a + H, bcol : bcol + W]
                        d = dpool.tile([128, FREE], cdt, tag="d")
                        d_v = d[:].rearrange(
                            "p (b h w) -> p b h w", b=HB, h=H, w=W
                        )
                        if k in DVE_PROD_KS:
                            nc.vector.tensor_scalar(
                                d_v, xwin, nbt[:, nt * 9 + k : nt * 9 + k + 1],
                                None, Alu.subtract,
                            )
                            nc.vector.tensor_scalar(
                                d[:].bitcast(idt), d[:].bitcast(idt), mask, None,
                                Alu.bitwise_and,
                            )
                        else:
                            nc.scalar.activation(
                                d_v, xwin, AF.Abs,
                                bias=nbneg[:, nt * 9 + k : nt * 9 + k + 1],
                                scale=1.0,
                            )
                        if first is None:
                            first = d
                        elif nfold == 0:
                            nc.vector.tensor_tensor(acc[:], first[:], d[:], Alu.max)
                            nfold = 1
                        else:
                            nc.vector.tensor_tensor(acc[:], acc[:], d[:], Alu.max)
                    acc_s = acc[:].rearrange("p (b s) -> p b s", b=HB)
                    dst = out_v[nt][:, h * HB : (h + 1) * HB]
                    if PREC == "fp16":
                        nc.gpsimd.dma_start(dst, acc_s)  # SWDGE cast fp16->f32
                    else:
                        nc.sync.dma_start(dst, acc_s)

    nc.compile()
    return nc


def _get_module():
    if "nc" not in _module_cache:
        _module_cache["nc"] = _build_module()
    return _module_cache["nc"]


def _run(x, neighbors, trace=False):
    from concourse import bass_utils

    x = np.ascontiguousarray(x, dtype=np.float32)
    neighbors = np.ascontiguousarray(neighbors, dtype=np.float32)
    in_maps = []
    for core in range(NCORES):
        bg, ng = divmod(core, NG)
        in_maps.append(
            {
                "x": x[bg * B_LOC : (bg + 1) * B_LOC],
                "neighbors": neighbors[ng * N_LOC : (ng + 1) * N_LOC],
            }
        )
    res = bass_utils.run_bass_kernel_spmd(
        _get_module(), in_maps, core_ids=list(range(NCORES)), trace=trace
    )
    out = np.empty((B, NUM * C, H, W), dtype=np.float32)
    for core in range(NCORES):
        bg, ng = divmod(core, NG)
        out[bg * B_LOC : (bg + 1) * B_LOC, ng * N_LOC * C : (ng + 1) * N_LOC * C] = (
            res.results[core]["out"]
        )
    return out, res


def kernel(x, neighbors):
    out, _ = _run(x, neighbors, trace=False)
    return out


# revision 17
# speedup vs baseline: 2.8200x; 1.0325x over previous
"""ConvNearestNeightbor Trainium2 kernel.

out[b, n*C+c, i, j] = max_k |x[b,c,i-r_k,j-c_k] - neighbors[n,c,k]|
over the 9 zero-padded 3x3 shifts (r_k, c_k).

Sharding: 8 cores = 4 batch-groups x 2 num-groups.
Per core: B_loc=4 batches, N_loc=16 codebook entries.
Partition layout: (nn in 0..3, c in 0..31) -> 128 partitions, with the
codebook tile index nt in 0..3 selecting n = nt*4+nn.
Work is split into batch-halves: each (nt, half) chain runs ops of
free size 2*32*32 = 2048 (b-pair, pixels).

Per chain: 9 abs-diff planes d_k = |x_win_k - nb_k| are produced
(ScalarE Abs activation with per-partition bias -nb for most k;
VectorE tensor_scalar subtract + bitwise-and sign clear for a few),
then folded with tensor_tensor max on VectorE.
PREC="fp16" keeps d/acc in fp16 (2x DVE fold rate, one fp16 rounding
~2^-11 relative); PREC="fp32" is bit-exact vs the fp32 reference.
"""

import numpy as np

B, C, H, W = 16, 32, 32, 32
NUM = 32
NCORES = 8
BG, NG = 4, 2          # batch groups x num groups
B_LOC = B // BG        # 4
N_LOC = NUM // NG      # 16
NT = N_LOC // 4        # 4 codebook tiles of 4 n each
PH, PW = H + 2, W + 2  # 34 x 34 padded image
HB = B_LOC // 2        # batches per half-chain
FREE = HB * H * W      # 2048 per half

PREC = "fp16"          # "fp16" or "fp32"
# shifts produced on VectorE (tensor_scalar subtract + bitwise-and sign
# clear); ScalarE (Abs+bias) takes the rest. Window offsets for DVE k's
# should be 4B-aligned for fp16 4x mode: k in {0,2,3,5,6,8}.
# Alternating 3/2 per chain balances ACT vs DVE totals.
DVE_PROD_EVEN = (0, 2, 6)
DVE_PROD_ODD = (0, 2)
# k processed first, reading the unpadded xraw tile directly (the center
# window is exactly the interior) so chains start before the pad-copy.
K_ORDER = (4, 0, 1, 2, 3, 5, 6, 7, 8)

_module_cache = {}


def _build_module():
    import concourse.bacc as bacc
    import concourse.mybir as mybir
    import concourse.tile as tile

    dt = mybir.dt
    Alu = mybir.AluOpType
    AF = mybir.ActivationFunctionType

    cdt = dt.float16 if PREC == "fp16" else dt.float32
    idt = dt.uint16 if PREC == "fp16" else dt.uint32
    mask = 0x7FFF if PREC == "fp16" else 0x7FFFFFFF

    nc = bacc.Bacc("TRN2", debug=False)
    x = nc.dram_tensor("x", [B_LOC, C, H, W], dt.float32, kind="ExternalInput")
    nb = nc.dram_tensor("neighbors", [N_LOC, C, 9], dt.float32, kind="ExternalInput")
    out = nc.dram_tensor(
        "out", [B_LOC, N_LOC * C, H, W], dt.float32, kind="ExternalOutput"
    )

    # window start offsets within the padded 34x34 image for the 9 shifts
    # k = (row+1)*3 + (col+1), window starts at (1-row, 1-col)
    offs = []
    for row in (-1, 0, 1):
        for col in (-1, 0, 1):
            offs.append((1 - row, 1 - col))

    with tile.TileContext(nc) as tc:
        with (
            tc.tile_pool(name="const", bufs=1) as cpool,
            tc.tile_pool(name="accp", bufs=4) as apool,
            tc.tile_pool(name="dp", bufs=8) as dpool,
        ):
            # raw x halves (contiguous loads on both HWDGE queues)
            x_src = x.ap().rearrange("b c h w -> c b h w")
            xraw = []
            for h in range(2):
                t = cpool.tile([128, FREE], dt.float32, tag=f"xraw{h}")
                tv = t[:].rearrange("p (b h w) -> p b h w", b=HB, h=H, w=W)
                for nn in range(4):
                    eng = nc.sync if nn % 2 == 0 else nc.scalar
                    eng.dma_start(
                        tv[nn * 32 : (nn + 1) * 32].rearrange("c b h w -> c b (h w)"),
                        x_src[:, h * HB : (h + 1) * HB].rearrange(
                            "c b h w -> c b (h w)"
                        ),
                    )
                xraw.append(tv)

            # padded halves, borders zero; interior cast-copied on ScalarE
            xpad = []
            for h in range(2):
                t = cpool.tile([128, HB * PH * PW], cdt, tag=f"xpad{h}")
                nc.gpsimd.memset(t[:], 0.0)
                tv = t[:].rearrange("p (b h w) -> p b h w", b=HB, h=PH, w=PW)
                nc.scalar.copy(tv[:, :, 1 : 1 + H, 1 : 1 + W], xraw[h])
                xpad.append(tv)

            nbt = cpool.tile([128, NT * 9], dt.float32, tag="nbt")
            # nbt[(nn,c), (t,k)] = neighbors[t*4+nn, c, k]
            nb_src = nb.ap().rearrange("(t nn) c k -> (nn c) t k", nn=4)
            nbt_v = nbt[:].rearrange("p (t k) -> p t k", t=NT)
            nc.gpsimd.dma_start(nbt_v, nb_src)
            # negated neighbors: ACT bias computes Abs(x + (-nb))
            nbneg = cpool.tile([128, NT * 9], dt.float32, tag="nbneg")
            nc.scalar.mul(nbneg[:], nbt[:], -1.0)

            # out[b, nt*128 + p, h, w] viewed per (nt, half)
            out_v = out.ap().rearrange("b (t p) h w -> t p b (h w)", t=NT)

            chain_idx = 0
            for nt in range(NT):
                for h in range(2):
                    dve_ks = DVE_PROD_EVEN if chain_idx % 2 == 0 else DVE_PROD_ODD
                    chain_idx += 1
                    acc = apool.tile([128, FREE], cdt, tag="acc")
                    nfold = 0
                    first = None
                    for k in K_ORDER:
                        a, bcol = offs[k]
                        if k == 4:
                            xwin = xraw[h]
                        else:
                            xwin = xpad[h][:, :, a : a + H, bcol : bcol + W]
                        d = dpool.tile([128, FREE], cdt, tag="d")
                        d_v = d[:].rearrange(
                            "p (b h w) -> p b h w", b=HB, h=H, w=W
                        )
                        if k in dve_ks:
                            nc.vector.tensor_scalar(
                                d_v, xwin, nbt[:, nt * 9 + k : nt * 9 + k + 1],
                                None, Alu.subtract,
                            )
                            nc.vector.tensor_scalar(
                                d[:].bitcast(idt), d[:].bitcast(idt), mask, None,
                                Alu.bitwise_and,
                            )
                        else:
                            nc.scalar.activation(
                                d_v, xwin, AF.Abs,
                                bias=nbneg[:, nt * 9 + k : nt * 9 + k + 1],
                                scale=1.0,
                            )
                        if first is None:
                            first = d
                        elif nfold == 0:
                            nc.vector.tensor_tensor(acc[:], first[:], d[:], Alu.max)
                            nfold = 1
                        else:
                            nc.vector.tensor_tensor(acc[:], acc[:], d[:], Alu.max)
                    acc_s = acc[:].rearrange("p (b s) -> p b s", b=HB)
                    dst = out_v[nt][:, h * HB : (h + 1) * HB]
                    if PREC == "fp16":
                        nc.gpsimd.dma_start(dst, acc_s)  # SWDGE cast fp16->f32
                    else:
                        nc.sync.dma_start(dst, acc_s)

    nc.compile()
    return nc


def _get_module():
    if "nc" not in _module_cache:
        _module_cache["nc"] = _build_module()
    return _module_cache["nc"]


def _run(x, neighbors, trace=False):
    from concourse import bass_utils

    x = np.ascontiguousarray(x, dtype=np.float32)
    neighbors = np.ascontiguousarray(neighbors, dtype=np.float32)
    in_maps = []
    for core in range(NCORES):
        bg, ng = divmod(core, NG)
        in_maps.append(
            {
                "x": x[bg * B_LOC : (bg + 1) * B_LOC],
                "neighbors": neighbors[ng * N_LOC : (ng + 1) * N_LOC],
            }
        )
    res = bass_utils.run_bass_kernel_spmd(
        _get_module(), in_maps, core_ids=list(range(NCORES)), trace=trace
    )
    out = np.empty((B, NUM * C, H, W), dtype=np.float32)
    for core in range(NCORES):
        bg, ng = divmod(core, NG)
        out[bg * B_LOC : (bg + 1) * B_LOC, ng * N_LOC * C : (ng + 1) * N_LOC * C] = (
            res.results[core]["out"]
        )
    return out, res


def kernel(x, neighbors):
    out, _ = _run(x, neighbors, trace=False)
    return out
